# revision 27
# baseline (speedup 1.0000x reference)
"""Multi-head attention (B=2, S=2048, E=1024, H=16, Dh=64) on 8 TRN2 NeuronCores.

Sharding: batch x head-group data/tensor parallel. Core c handles batch c//4
and heads [4*(c%4), 4*(c%4)+4): it computes Q/K/V projections for its 256
feature columns, full attention for its 4 heads, and a partial output
projection against its 256 rows of W_o. The host sums the 4 partials per
batch (the "all-reduce after W_o" step of the sharding hint, done at
unshard time) and concatenates the two batches.

Numerics (same as the validated v1): the pre-softmax path runs in float32r
(~2^-12 per-element input rounding, fp32 accumulate) at full PE rate. The
row max m comes from a q-major f32r score pass max-reduced on DVE; the
k-major score matmul subtracts m via an augmented contraction row (kT row
64 = 1, qT row 64 = -m), so exp() fuses the PSUM->SBUF copy on ScalarE
with scale=1/sqrt(Dh). The softmax denominator comes free from an appended
ones-column on V; normalization divides by the Pool-broadcast denominator
after the P@V matmul. P is fp16, V/att/W_o are f32r.

Schedule (v2, engine-balanced): PE is the roofline (~220us of matmuls);
every other engine is kept strictly below PE's per-slot pace so PE never
waits.
  - DVE runs ONLY the stats max-reduction, as wide [128,1024] PSUM reads
    (2 banks per reduce; ~9.8us per slot vs ~11us of PE work).
  - Pool (gpsimd) takes everything else elementwise: proj PSUM->SBUF
    staging copies, W_o output copies, the 1/l broadcast
    (partition_broadcast) and the normalize divide.
  - Slots iterate q-chunk OUTER, head INNER, so each chunk's W_o partial
    drains as fill work in the next chunk's slots (no W_o pileup at the
    tail).
  - Fills run two slots ahead (slot s drains stats matmuls for slot s+2
    and the aug-row transpose/DMA for slot s+1), so the aug DMA latency
    and the DVE reduce latency are fully hidden.
  - DMA order on the serialized DMA resource: wks, wvs, x0, x1, wqs, x2,
    x3, wos; kT/qT unstack DMAs ride the scalar queue, x/out ride sync,
    weights/aug ride the Pool SWDGE queue.
"""

from contextlib import ExitStack

import numpy as np

import concourse.bacc as bacc
import concourse.mybir as mybir
import concourse.tile as tile
from concourse import bass_utils
from concourse.masks import make_identity

AF = mybir.ActivationFunctionType
ALU = mybir.AluOpType
F32 = mybir.dt.float32
F16 = mybir.dt.float16
F32R = mybir.dt.float32r

B, S, E, H, Dh = 2, 2048, 1024, 16, 64
NCORES = 8
GROUPS = 4            # head groups (cores per batch)
HPC = H // GROUPS     # heads per core = 4
FG = HPC * Dh         # feature columns per core = 256
P = 128
SCALE = 1.0 / (Dh ** 0.5)

EO = E // P           # 8 contraction chunks
ST = S // P           # 16 sequence tiles of 128
QC = 512              # q-chunk width
NQC = S // QC         # 4
LAG = 5               # PV lags scores by LAG kt tiles


def _emit_v2(tc):
    nc = tc.nc
    xt = nc.dram_tensor("xt", [E, S], F32R, kind="ExternalInput").ap()
    wq = nc.dram_tensor("wq", [E, FG], F32R, kind="ExternalInput").ap()
    wk = nc.dram_tensor("wk", [E, FG], F32R, kind="ExternalInput").ap()
    wv = nc.dram_tensor("wv", [E, FG], F32R, kind="ExternalInput").ap()
    wo = nc.dram_tensor("wo", [FG, E], F32R, kind="ExternalInput").ap()
    out = nc.dram_tensor("out", [S, E], F32, kind="ExternalOutput").ap()

    ctx = ExitStack()
    const = ctx.enter_context(tc.tile_pool(name="const", bufs=1))
    persist = ctx.enter_context(tc.tile_pool(name="persist", bufs=1))
    stage = ctx.enter_context(tc.tile_pool(name="stage", bufs=3))
    xqp = ctx.enter_context(tc.tile_pool(name="xqp", bufs=3))
    stgp = ctx.enter_context(tc.tile_pool(name="stgp", bufs=3))
    ptp = ctx.enter_context(tc.tile_pool(name="ptp", bufs=4))
    obp = ctx.enter_context(tc.tile_pool(name="obp", bufs=4))
    stage2 = ctx.enter_context(tc.tile_pool(name="stage2", bufs=2))
    mxp = ctx.enter_context(tc.tile_pool(name="mxp", bufs=4))
    hmp = ctx.enter_context(tc.tile_pool(name="hmp", bufs=10))
    ps_st = ctx.enter_context(tc.tile_pool(name="ps_st", bufs=1, space="PSUM"))
    ps_sc = ctx.enter_context(tc.tile_pool(name="ps_sc", bufs=2, space="PSUM"))
    ps_pv = ctx.enter_context(tc.tile_pool(name="ps_pv", bufs=1, space="PSUM"))
    ps_wo = ctx.enter_context(tc.tile_pool(name="ps_wo", bufs=1, space="PSUM"))

    ones_f32 = const.tile([P, Dh], F32)
    nc.gpsimd.memset(ones_f32[:], 1.0)
    ones_mat = const.tile([P, Dh], F32R)
    nc.vector.tensor_copy(ones_mat[:], ones_f32[:])
    ident = const.tile([P, P], F32)
    make_identity(nc, ident[:])


    wqs = persist.tile([P, EO, FG], F32R)
    wks = persist.tile([P, EO, FG], F32R)
    wvs = persist.tile([P, EO, FG], F32R)
    wos = persist.tile([P, FG // P, E], F32R)
    qT = persist.tile([P, HPC, S], F32R)
    kT = persist.tile([P, HPC, S], F32R)
    vau = persist.tile([P, ST, HPC, Dh + 1], F16)
    att = persist.tile([P, FG // P, S], F32R)

    xt_re = xt.rearrange("(eo p) s -> p eo s", p=P)

    # DMA order on the serialized DMA resource: wks/x0 halves interleaved so
    # K proj starts ~6us in, then x1..x3 with just-in-time weights between.
    wk_re = wk.rearrange("(eo p) m -> p eo m", p=P)
    xq_tiles = {}
    for qc4 in range(3):
        xq_tiles[qc4] = xqp.tile([P, EO, QC], F32R, tag="xq", name=f"xq{qc4}")
    nc.sync.dma_start(wks[:, 0:4, :], wk_re[:, 0:4, :])
    nc.sync.dma_start(xq_tiles[0][:, :, 0:256], xt_re[:, :, 0:256])
    nc.sync.dma_start(wks[:, 4:, :], wk_re[:, 4:, :])
    nc.sync.dma_start(xq_tiles[0][:, :, 256:512], xt_re[:, :, 256:512])
    nc.sync.dma_start(wvs[:], wv.rearrange("(eo p) m -> p eo m", p=P))
    nc.sync.dma_start(xq_tiles[1][:, :, 0:256], xt_re[:, :, QC : QC + 256])
    nc.sync.dma_start(xq_tiles[1][:, :, 256:512], xt_re[:, :, QC + 256 : 2 * QC])
    nc.sync.dma_start(wqs[:], wq.rearrange("(eo p) m -> p eo m", p=P))
    xq3 = xqp.tile([P, EO, QC], F32R, tag="xq")
    xq_tiles[3] = xq3
    nc.sync.dma_start(xq_tiles[2][:], xt_re[:, :, 2 * QC : 3 * QC])


    def emit_x3():
        nc.sync.dma_start(xq3[:], xt_re[:, :, 3 * QC : 4 * QC])

    def emit_wos():
        nc.sync.dma_start(wos[:], wo.rearrange("(fo p) e -> p fo e", p=P))

    # PE p-state warmup: dummy matmuls on const data while the first x
    # chunk is in flight (results discarded; psum overwritten later).
    wmps = ps_st.tile([P, 1024], F32, tag="st", name="warm_ps")
    for _ in range(24):
        nc.tensor.matmul(
            wmps[0:Dh, 0:Dh], lhsT=ones_mat[0:P, :],
            rhs=ones_f32[:].bitcast(F32R),
            start=True, stop=True, skip_group_check=True,
        )
    nc.gpsimd.memset(kT[Dh : Dh + 1, :, :].bitcast(F32), 1.0)
    nc.gpsimd.memset(vau[:, :, :, Dh : Dh + 1], 1.0)
    warm = stage.tile([P, 4], F32, tag="warm")
    nc.gpsimd.memset(warm[:], 0.0)
    warm2 = stage.tile([P, 4], F16, tag="warm2")
    nc.scalar.activation(warm2[:], warm[:], AF.Exp, scale=1.0)

    # ---------- projections ----------
    def kproj(pp, qc4, split_first=False):
        xq = xq_tiles[qc4]
        qs = slice(qc4 * QC, (qc4 + 1) * QC)
        for mc in range(FG // P):
            stg = stgp.tile([P, QC], F32R, tag="stg")
            if split_first and mc == 0:
                ps = pp.tile([P, QC], F32, tag="sc", name="ps_k0")[:, :256]
                for half in range(2):
                    cs = slice(half * 256, (half + 1) * 256)
                    for eo in range(EO):
                        nc.tensor.matmul(
                            ps,
                            lhsT=wks[:, eo, mc * P : (mc + 1) * P],
                            rhs=xq[:, eo, cs],
                            start=(eo == 0),
                            stop=(eo == EO - 1),
                        )
                    nc.vector.tensor_copy(stg[:, cs], ps)
            else:
                ps = pp.tile([P, QC], F32, tag="sc", name="ps_k")
                for eo in range(EO):
                    nc.tensor.matmul(
                        ps,
                        lhsT=wks[:, eo, mc * P : (mc + 1) * P],
                        rhs=xq[:, eo, :],
                        start=(eo == 0),
                        stop=(eo == EO - 1),
                    )
                nc.vector.tensor_copy(stg[:], ps)
            for hh in range(2):
                h = mc * 2 + hh
                nc.sync.dma_start(kT[0:Dh, h, qs], stg[hh * Dh : (hh + 1) * Dh, :])

    def vproj(pp, qc4):
        xq = xq_tiles[qc4]
        for st4 in range(4):
            st = qc4 * 4 + st4
            ps = pp.tile([P, QC], F32, tag="sc", name="ps_v")[:, :FG]
            for eo in range(EO):
                nc.tensor.matmul(
                    ps,
                    lhsT=xq[:, eo, st4 * P : (st4 + 1) * P],
                    rhs=wvs[:, eo, :],
                    start=(eo == 0),
                    stop=(eo == EO - 1),
                )
            nc.scalar.copy(
                vau[:, st, :, 0:Dh],
                ps.rearrange("p (h d) -> p h d", h=HPC),
            )

    def qproj(pp, qc4):
        for step in qproj_steps(pp, qc4):
            step()

    def qproj_steps(pp, qc4):
        xq = xq_tiles[qc4]
        qs = slice(qc4 * QC, (qc4 + 1) * QC)
        for mc in range(FG // P):
            ps = pp.tile([P, QC], F32, tag="sc", name="ps_q")
            for eo in range(EO):

                def mm(ps=ps, mc=mc, eo=eo, xq=xq, qs=qs, last=(eo == EO - 1)):
                    nc.tensor.matmul(
                        ps,
                        lhsT=wqs[:, eo, mc * P : (mc + 1) * P],
                        rhs=xq[:, eo, :],
                        start=(eo == 0),
                        stop=last,
                    )
                    if last:
                        stg = stgp.tile([P, QC], F32R, tag="stg")
                        nc.vector.tensor_copy(stg[:], ps)
                        for hh in range(2):
                            h = mc * 2 + hh
                            nc.sync.dma_start(
                                qT[0:Dh, h, qs], stg[hh * Dh : (hh + 1) * Dh, :]
                            )

                yield mm

    # ---------- stats (q-major max pass) ----------
    mx_tiles = {}

    def stats_steps(h, qc4, pair_major=False):
        """16 PE steps; DVE wide reduces attached; fills mx_tiles[(h,qc4)].

        pair_major=True emits all kc01 pairs before any kc23 pair, so the
        early reduces only depend on the first half of kT (overlaps the
        tail of the projection phase)."""
        mx = mxp.tile([P, 4], F32, tag="mx", name=f"mx{h}_{qc4}")
        mx_tiles[(h, qc4)] = mx
        hms = {}
        order = ([(qt4, pair) for pair in range(2) for qt4 in range(4)]
                 if pair_major else
                 [(qt4, pair) for qt4 in range(4) for pair in range(2)])
        for qt4, pair in order:
            qt = qc4 * 4 + qt4
            if qt4 not in hms:
                hms[qt4] = hmp.tile([P, 2], F32, tag="hm", name=f"hm{h}_{qc4}_{qt4}")
            hm = hms[qt4]
            st_t = ps_st.tile([P, 1024], F32, tag="st", name="ps_stat")
            for j in range(2):
                kc = pair * 2 + j

                def mm(st_t=st_t, j=j, kc=kc, h=h, qt=qt, pair=pair,
                       hm=hm, qt4=qt4, mx=mx, last=(j == 1)):
                    nc.tensor.matmul(
                        st_t[:, j * QC : (j + 1) * QC],
                        lhsT=qT[0:Dh, h, qt * P : (qt + 1) * P],
                        rhs=kT[0:Dh, h, kc * QC : (kc + 1) * QC],
                        start=True,
                        stop=True,
                    )
                    if last:
                        nc.vector.tensor_reduce(
                            hm[:, pair : pair + 1], st_t[:],
                            axis=mybir.AxisListType.X, op=ALU.max,
                        )
                        if pair == 1:
                            nc.vector.tensor_reduce(
                                mx[:, qt4 : qt4 + 1], hm[:, 0:2],
                                axis=mybir.AxisListType.X, op=ALU.max,
                            )

                yield mm

    def drain(it, n=1 << 30):
        k = 0
        if it is not None:
            for step in it:
                step()
                k += 1
                if k >= n:
                    break

    def aug_steps(ps_pool, h, qc4):
        def step():
            mx = mx_tiles[(h, qc4)]
            psm = ps_pool.tile([P, QC], F32, tag="wo", name="psm")
            nc.tensor.transpose(psm[0:4, 0:P], mx[:, :], ident[:])
            mst = stage.tile([4, P], F32R, tag="mst")
            nc.scalar.mul(mst[:], psm[0:4, 0:P], -1.0)
            nc.gpsimd.dma_start(
                qT[Dh : Dh + 1, h, qc4 * QC : (qc4 + 1) * QC], mst[:, :]
            )
        yield step

    # ---------- W_o drain for one chunk ----------
    def wo_steps(ps_pools, qc4, alternate_ob=False):
        i = 0
        for qt4 in range(4):
            qt = qc4 * 4 + qt4
            for ec in range(E // QC):
                pool, ptag = ps_pools[i % len(ps_pools)]
                ps = pool.tile([P, QC], F32, tag=ptag, name="ps_wo")
                use_dve = alternate_ob and (i % 2 == 1)
                i += 1
                for fc in range(FG // P):

                    def mm(ps=ps, qt=qt, ec=ec, fc=fc, use_dve=use_dve,
                           last=(fc == FG // P - 1)):
                        nc.tensor.matmul(
                            ps,
                            lhsT=att[:, fc, qt * P : (qt + 1) * P],
                            rhs=wos[:, fc, ec * QC : (ec + 1) * QC],
                            start=(fc == 0),
                            stop=last,
                            skip_group_check=True,
                        )
                        if last:
                            ob = obp.tile([P, QC], F32, tag="ob")
                            if use_dve:
                                nc.vector.tensor_copy(ob[:], ps)
                            else:
                                nc.scalar.copy(ob[:], ps)
                            nc.sync.dma_start(
                                out[qt * P : (qt + 1) * P, ec * QC : (ec + 1) * QC],
                                ob[:],
                            )

                    yield mm

    slots = [(c, h) for c in range(NQC) for h in range(HPC)]
    NSLOT = len(slots)

    pp = ps_sc
    kproj(pp, 0, split_first=True)
    vproj(pp, 0)
    qproj(pp, 0)
    kproj(pp, 1, split_first=True)
    emit_x3()
    vproj(pp, 1)
    kproj(pp, 2)
    vproj(pp, 2)
    kproj(pp, 3)
    vproj(pp, 3)
    emit_wos()
    def chain(gens):
        for g in gens:
            yield from g

    # pre-slot: stats for slots 0 and 1, with Q1/Q2 projections as PE
    # filler while DVE grinds through the wide reduces (2-buf st pool)
    g0 = stats_steps(slots[0][1], slots[0][0], pair_major=True)
    g1 = stats_steps(slots[1][1], slots[1][0], pair_major=True)
    gq = chain([qproj_steps(ps_sc, 1), qproj_steps(ps_sc, 2),
                qproj_steps(ps_sc, 3)])
    for _ in range(8):
        drain(g0, 2)
        drain(gq, 3)
    for _ in range(8):
        drain(g1, 2)
        drain(gq, 3)
    drain(g0)
    drain(g1)
    drain(gq)

    # aug row for slot 0 must land before its first score matmul
    drain(aug_steps(ps_wo, slots[0][1], slots[0][0]))

    # fill lists per slot: [aug(s+1)] + stats(s+2) + wo drip
    fills = []
    for s in range(NSLOT):
        f = []
        aug_f = None
        if s + 1 < NSLOT:
            c1, h1 = slots[s + 1]
            aug_f = aug_steps(ps_wo, h1, c1)
        if s + 2 < NSLOT:
            c2, h2 = slots[s + 2]
            f.append(stats_steps(h2, c2))
        fills.append((aug_f, f))
    # wo drip: wo(c) spread over the 4 slots of group c+1
    wo_gens = {}
    for c in range(NQC - 1):
        wo_gens[c] = wo_steps([(ps_wo, "wo")], c)

    def norm_steps(pv, li, h, qs):
        # deferred normalize: pb broadcast matmul, Act copy to SBUF, DVE mult.
        # Drained a few kt into the NEXT slot so the pb matmul never waits on
        # the reciprocal at the head of PE's queue.
        def step():
            pb = ps_wo.tile([P, QC], F32, tag="wo", name="pb")
            nc.tensor.matmul(
                pb[0:Dh, :], lhsT=ones_mat[Dh : Dh + 1, :],
                rhs=li[Dh : Dh + 1, :], start=True, stop=True,
            )
            bc = stage2.tile([P, QC], F32, tag="bc")
            nc.scalar.copy(bc[0:Dh, :], pb[0:Dh, :])
            with nc.allow_low_precision(reason="normalize in f32r is ample"):
                if h % 2 == 0:
                    nc.vector.tensor_tensor(
                        att[0:Dh, h // 2, qs], pv[0:Dh, :], bc[0:Dh, :], ALU.mult
                    )
                else:
                    stg = stage2.tile([P, QC], F32R, tag="att_stg")
                    nc.vector.tensor_tensor(
                        stg[0:Dh, :], pv[0:Dh, :], bc[0:Dh, :], ALU.mult
                    )
                    nc.gpsimd.dma_start(
                        att[Dh : 2 * Dh, h // 2, qs], stg[0:Dh, :]
                    )
        yield step

    pending_norm = None
    for s, (qc4, h) in enumerate(slots):
        qs = slice(qc4 * QC, (qc4 + 1) * QC)
        aug_f, f_list = fills[s]
        fill = chain(f_list)
        aug_kt = 4 if s == 0 else 1
        # wo drip comes from the previous chunk's generator; for the first
        # slot of a group, delay until the previous group's last norm landed
        wo_g = wo_gens.get(qc4 - 1)
        wo_kts = (5, 9, 13, 15) if h == 0 else (1, 5, 9, 13)

        pv = ps_pv.tile([P, QC], F32, tag="pv")
        pts = {}

        def pv_mm(kt, pv=pv, h=h, pts=pts):
            nc.tensor.matmul(
                pv[0 : Dh + 1, :],
                lhsT=vau[:, kt, h, :],
                rhs=pts.pop(kt),
                start=(kt == 0),
                stop=(kt == ST - 1),
                skip_group_check=True,
            )

        sc2 = None
        for kt in range(ST):
            ks = slice(kt * P, (kt + 1) * P)
            if kt == 1 and pending_norm is not None:
                drain(pending_norm)
                pending_norm = None
            if kt == aug_kt and aug_f is not None:
                drain(aug_f)
            # stats fills are strictly paced (single st buf, DVE-bound);
            # other fills are front-loaded
            drain(fill, 2 if kt < 8 else 1)
            if wo_g is not None and kt in wo_kts:
                drain(wo_g, 1)
            if kt % 2 == 0:
                sc2 = ps_sc.tile([P, 2, QC], F32, tag="sc")
            nc.tensor.matmul(
                sc2[:, kt % 2, :], lhsT=kT[0 : Dh + 1, h, ks],
                rhs=qT[0 : Dh + 1, h, qs], start=True, stop=True,
            )
            if kt % 2 == 1:
                pt2 = ptp.tile([P, 2, QC], F16, tag="pt")
                pts[kt - 1] = pt2[:, 0, :]
                pts[kt] = pt2[:, 1, :]
                nc.scalar.activation(pt2[:], sc2[:], AF.Exp, scale=SCALE)
            if kt >= LAG:
                pv_mm(kt - LAG)
        drain(fill)
        for kt in range(ST - LAG, ST):
            pv_mm(kt)

        li = stage2.tile([P, QC], F32R, tag="li")
        with nc.allow_low_precision(reason="1/l in f32r (~2^-12) is ample"):
            nc.vector.reciprocal(li[Dh : Dh + 1, :], pv[Dh : Dh + 1, :])
        pending_norm = norm_steps(pv, li, h, qs)

    # tail: last slot's norm, then the last chunk's W_o with ob copies
    # alternating Act/DVE (both engines idle by now)
    drain(pending_norm)
    drain(wo_steps([(ps_wo, "wo"), (ps_sc, "sc")], NQC - 1, alternate_ob=True))
    ctx.close()


_NC = None


def _build():
    global _NC
    if _NC is None:
        nc = bacc.Bacc(
            "TRN2", target_bir_lowering=False, debug=False, num_devices=NCORES
        )
        with tile.TileContext(nc) as tc:
            _emit_v2(tc)
        nc.compile()
        _NC = nc
    return _NC


def _prep_inputs(x, W_q, W_k, W_v, W_o):
    x = np.asarray(x, dtype=np.float32)
    W_q = np.asarray(W_q, dtype=np.float32)
    W_k = np.asarray(W_k, dtype=np.float32)
    W_v = np.asarray(W_v, dtype=np.float32)
    W_o = np.asarray(W_o, dtype=np.float32)

    xts = [np.ascontiguousarray(x[b].T) for b in range(B)]
    in_maps = []
    for c in range(NCORES):
        b, g = divmod(c, GROUPS)
        fg = slice(g * FG, (g + 1) * FG)
        in_maps.append(
            {
                "xt": xts[b],
                "wq": np.ascontiguousarray(W_q[:, fg]),
                "wk": np.ascontiguousarray(W_k[:, fg]),
                "wv": np.ascontiguousarray(W_v[:, fg]),
                "wo": np.ascontiguousarray(W_o[fg, :]),
            }
        )
    return in_maps


def run(inputs, **spmd_kwargs):
    nc = _build()
    in_maps = _prep_inputs(
        inputs["x"], inputs["W_q"], inputs["W_k"], inputs["W_v"], inputs["W_o"]
    )
    res = bass_utils.run_bass_kernel_spmd(
        nc, in_maps, core_ids=list(range(NCORES)), **spmd_kwargs
    )
    out = np.zeros((B, S, E), dtype=np.float32)
    for c in range(NCORES):
        out[c // GROUPS] += res.results[c]["out"]
    return out, res


def kernel(**inputs):
    out, _ = run(inputs)
    return out


# revision 28
# speedup vs baseline: 1.1139x; 1.1139x over previous
"""Multi-head attention (B=2, S=2048, E=1024, H=16, Dh=64) on 8 TRN2 NeuronCores.

Sharding: batch x head-group data/tensor parallel. Core c handles batch c//4
and heads [4*(c%4), 4*(c%4)+4): it computes Q/K/V projections for its 256
feature columns, full attention for its 4 heads, and a partial output
projection against its 256 rows of W_o. The host sums the 4 partials per
batch (the "all-reduce after W_o" step of the sharding hint, done at
unshard time) and concatenates the two batches.

Numerics (same as the validated v1): the pre-softmax path runs in float32r
(~2^-12 per-element input rounding, fp32 accumulate) at full PE rate. The
row max m comes from a q-major f32r score pass max-reduced on DVE; the
k-major score matmul subtracts m via an augmented contraction row (kT row
64 = 1, qT row 64 = -m), so exp() fuses the PSUM->SBUF copy on ScalarE
with scale=1/sqrt(Dh). The softmax denominator comes free from an appended
ones-column on V; normalization divides by the Pool-broadcast denominator
after the P@V matmul. P is fp16, V/att/W_o are f32r.

Schedule (v2, engine-balanced): PE is the roofline (~220us of matmuls);
every other engine is kept strictly below PE's per-slot pace so PE never
waits.
  - DVE runs ONLY the stats max-reduction, as wide [128,1024] PSUM reads
    (2 banks per reduce; ~9.8us per slot vs ~11us of PE work).
  - Pool (gpsimd) takes everything else elementwise: proj PSUM->SBUF
    staging copies, W_o output copies, the 1/l broadcast
    (partition_broadcast) and the normalize divide.
  - Slots iterate q-chunk OUTER, head INNER, so each chunk's W_o partial
    drains as fill work in the next chunk's slots (no W_o pileup at the
    tail).
  - Fills run two slots ahead (slot s drains stats matmuls for slot s+2
    and the aug-row transpose/DMA for slot s+1), so the aug DMA latency
    and the DVE reduce latency are fully hidden.
  - DMA order on the serialized DMA resource: wks, wvs, x0, x1, wqs, x2,
    x3, wos; kT/qT unstack DMAs ride the scalar queue, x/out ride sync,
    weights/aug ride the Pool SWDGE queue.
"""

from contextlib import ExitStack

import numpy as np

import concourse.bacc as bacc
import concourse.mybir as mybir
import concourse.tile as tile
from concourse import bass_utils
from concourse.masks import make_identity

AF = mybir.ActivationFunctionType
ALU = mybir.AluOpType
F32 = mybir.dt.float32
F16 = mybir.dt.float16
F32R = mybir.dt.float32r

B, S, E, H, Dh = 2, 2048, 1024, 16, 64
NCORES = 8
GROUPS = 4            # head groups (cores per batch)
HPC = H // GROUPS     # heads per core = 4
FG = HPC * Dh         # feature columns per core = 256
P = 128
SCALE = 1.0 / (Dh ** 0.5)

EO = E // P           # 8 contraction chunks
ST = S // P           # 16 sequence tiles of 128
QC = 512              # q-chunk width
NQC = S // QC         # 4
LAG = 5               # PV lags scores by LAG kt tiles


def _emit_v2(tc):
    nc = tc.nc
    xt = nc.dram_tensor("xt", [E, S], F32R, kind="ExternalInput").ap()
    wq = nc.dram_tensor("wq", [E, FG], F32R, kind="ExternalInput").ap()
    wk = nc.dram_tensor("wk", [E, FG], F32R, kind="ExternalInput").ap()
    wv = nc.dram_tensor("wv", [E, FG], F32R, kind="ExternalInput").ap()
    wo = nc.dram_tensor("wo", [FG, E], F32R, kind="ExternalInput").ap()
    out = nc.dram_tensor("out", [S, E], F32, kind="ExternalOutput").ap()

    ctx = ExitStack()
    const = ctx.enter_context(tc.tile_pool(name="const", bufs=1))
    persist = ctx.enter_context(tc.tile_pool(name="persist", bufs=1))
    stage = ctx.enter_context(tc.tile_pool(name="stage", bufs=3))
    xqp = ctx.enter_context(tc.tile_pool(name="xqp", bufs=3))
    stgp = ctx.enter_context(tc.tile_pool(name="stgp", bufs=3))
    ptp = ctx.enter_context(tc.tile_pool(name="ptp", bufs=4))
    obp = ctx.enter_context(tc.tile_pool(name="obp", bufs=4))
    stage2 = ctx.enter_context(tc.tile_pool(name="stage2", bufs=2))
    mxp = ctx.enter_context(tc.tile_pool(name="mxp", bufs=4))
    hmp = ctx.enter_context(tc.tile_pool(name="hmp", bufs=10))
    ps_sc = ctx.enter_context(tc.tile_pool(name="ps_sc", bufs=3, space="PSUM"))
    ps_pv = ctx.enter_context(tc.tile_pool(name="ps_pv", bufs=1, space="PSUM"))
    ps_wo = ctx.enter_context(tc.tile_pool(name="ps_wo", bufs=1, space="PSUM"))

    ones_f32 = const.tile([P, Dh], F32)
    nc.gpsimd.memset(ones_f32[:], 1.0)
    ones_mat = const.tile([P, Dh], F32R)
    nc.vector.tensor_copy(ones_mat[:], ones_f32[:])
    ident = const.tile([P, P], F32)
    make_identity(nc, ident[:])


    wqs = persist.tile([P, EO, FG], F32R)
    wks = persist.tile([P, EO, FG], F32R)
    wvs = persist.tile([P, EO, FG], F32R)
    wos = persist.tile([P, FG // P, E], F32R)
    qT = persist.tile([P, HPC, S], F32R)
    kT = persist.tile([P, HPC, S], F32R)
    vau = persist.tile([P, ST, HPC, Dh + 1], F16)
    att = persist.tile([P, FG // P, S], F32R)

    xt_re = xt.rearrange("(eo p) s -> p eo s", p=P)

    # DMA order on the serialized DMA resource: wks/x0 halves interleaved so
    # K proj starts ~6us in, then x1..x3 with just-in-time weights between.
    wk_re = wk.rearrange("(eo p) m -> p eo m", p=P)
    xq_tiles = {}
    for qc4 in range(3):
        xq_tiles[qc4] = xqp.tile([P, EO, QC], F32R, tag="xq", name=f"xq{qc4}")
    nc.sync.dma_start(wks[:, 0:4, :], wk_re[:, 0:4, :])
    nc.sync.dma_start(xq_tiles[0][:, :, 0:256], xt_re[:, :, 0:256])
    nc.sync.dma_start(wks[:, 4:, :], wk_re[:, 4:, :])
    nc.sync.dma_start(xq_tiles[0][:, :, 256:512], xt_re[:, :, 256:512])
    nc.sync.dma_start(wvs[:], wv.rearrange("(eo p) m -> p eo m", p=P))
    nc.sync.dma_start(xq_tiles[1][:, :, 0:256], xt_re[:, :, QC : QC + 256])
    nc.sync.dma_start(xq_tiles[1][:, :, 256:512], xt_re[:, :, QC + 256 : 2 * QC])
    nc.sync.dma_start(wqs[:], wq.rearrange("(eo p) m -> p eo m", p=P))
    xq3 = xqp.tile([P, EO, QC], F32R, tag="xq")
    xq_tiles[3] = xq3
    nc.sync.dma_start(xq_tiles[2][:], xt_re[:, :, 2 * QC : 3 * QC])


    def emit_x3():
        nc.sync.dma_start(xq3[:], xt_re[:, :, 3 * QC : 4 * QC])

    def emit_wos():
        nc.sync.dma_start(wos[:], wo.rearrange("(fo p) e -> p fo e", p=P))

    # PE p-state warmup: dummy matmuls on const data while the first x
    # chunk is in flight (results discarded; psum overwritten later).
    wmps = ps_sc.tile([P, 2, QC], F32, tag="sc", name="warm_ps")
    for _ in range(24):
        nc.tensor.matmul(
            wmps[0:Dh, 0, 0:Dh], lhsT=ones_mat[0:P, :],
            rhs=ones_f32[:].bitcast(F32R),
            start=True, stop=True, skip_group_check=True,
        )
    nc.gpsimd.memset(kT[Dh : Dh + 1, :, :].bitcast(F32), 1.0)
    nc.gpsimd.memset(vau[:, :, :, Dh : Dh + 1], 1.0)
    warm = stage.tile([P, 4], F32, tag="warm")
    nc.gpsimd.memset(warm[:], 0.0)
    warm2 = stage.tile([P, 4], F16, tag="warm2")
    nc.scalar.activation(warm2[:], warm[:], AF.Exp, scale=1.0)

    # ---------- projections ----------
    def kproj(pp, qc4, split_first=False):
        xq = xq_tiles[qc4]
        qs = slice(qc4 * QC, (qc4 + 1) * QC)
        for mc in range(FG // P):
            stg = stgp.tile([P, QC], F32R, tag="stg")
            if split_first and mc == 0:
                ps = pp.tile([P, QC], F32, tag="sc", name="ps_k0")[:, :256]
                for half in range(2):
                    cs = slice(half * 256, (half + 1) * 256)
                    for eo in range(EO):
                        nc.tensor.matmul(
                            ps,
                            lhsT=wks[:, eo, mc * P : (mc + 1) * P],
                            rhs=xq[:, eo, cs],
                            start=(eo == 0),
                            stop=(eo == EO - 1),
                        )
                    nc.vector.tensor_copy(stg[:, cs], ps)
            else:
                ps = pp.tile([P, QC], F32, tag="sc", name="ps_k")
                for eo in range(EO):
                    nc.tensor.matmul(
                        ps,
                        lhsT=wks[:, eo, mc * P : (mc + 1) * P],
                        rhs=xq[:, eo, :],
                        start=(eo == 0),
                        stop=(eo == EO - 1),
                    )
                nc.vector.tensor_copy(stg[:], ps)
            for hh in range(2):
                h = mc * 2 + hh
                nc.sync.dma_start(kT[0:Dh, h, qs], stg[hh * Dh : (hh + 1) * Dh, :])

    def vproj(pp, qc4):
        xq = xq_tiles[qc4]
        for st4 in range(4):
            st = qc4 * 4 + st4
            ps = pp.tile([P, QC], F32, tag="sc", name="ps_v")[:, :FG]
            for eo in range(EO):
                nc.tensor.matmul(
                    ps,
                    lhsT=xq[:, eo, st4 * P : (st4 + 1) * P],
                    rhs=wvs[:, eo, :],
                    start=(eo == 0),
                    stop=(eo == EO - 1),
                )
            nc.scalar.copy(
                vau[:, st, :, 0:Dh],
                ps.rearrange("p (h d) -> p h d", h=HPC),
            )

    def qproj(pp, qc4):
        for step in qproj_steps(pp, qc4):
            step()

    def qproj_steps(pp, qc4):
        xq = xq_tiles[qc4]
        qs = slice(qc4 * QC, (qc4 + 1) * QC)
        for mc in range(FG // P):
            ps = pp.tile([P, QC], F32, tag="sc", name="ps_q")
            for eo in range(EO):

                def mm(ps=ps, mc=mc, eo=eo, xq=xq, qs=qs, last=(eo == EO - 1)):
                    nc.tensor.matmul(
                        ps,
                        lhsT=wqs[:, eo, mc * P : (mc + 1) * P],
                        rhs=xq[:, eo, :],
                        start=(eo == 0),
                        stop=last,
                    )
                    if last:
                        stg = stgp.tile([P, QC], F32R, tag="stg")
                        nc.vector.tensor_copy(stg[:], ps)
                        for hh in range(2):
                            h = mc * 2 + hh
                            nc.sync.dma_start(
                                qT[0:Dh, h, qs], stg[hh * Dh : (hh + 1) * Dh, :]
                            )

                yield mm

    # ---------- stats (q-major max pass) ----------
    mx_tiles = {}

    def stats_steps(h, qc4, pair_major=False):
        """16 PE steps; DVE wide reduces attached; fills mx_tiles[(h,qc4)].

        pair_major=True emits all kc01 pairs before any kc23 pair, so the
        early reduces only depend on the first half of kT (overlaps the
        tail of the projection phase)."""
        mx = mxp.tile([P, 4], F32, tag="mx", name=f"mx{h}_{qc4}")
        mx_tiles[(h, qc4)] = mx
        hms = {}
        order = ([(qt4, pair) for pair in range(2) for qt4 in range(4)]
                 if pair_major else
                 [(qt4, pair) for qt4 in range(4) for pair in range(2)])
        for qt4, pair in order:
            qt = qc4 * 4 + qt4
            if qt4 not in hms:
                hms[qt4] = hmp.tile([P, 2], F32, tag="hm", name=f"hm{h}_{qc4}_{qt4}")
            hm = hms[qt4]
            st_t = ps_sc.tile([P, 2, QC], F32, tag="sc", name="ps_stat")
            for j in range(2):
                kc = pair * 2 + j

                def mm(st_t=st_t, j=j, kc=kc, h=h, qt=qt, pair=pair,
                       hm=hm, qt4=qt4, mx=mx, last=(j == 1)):
                    nc.tensor.matmul(
                        st_t[:, j, :],
                        lhsT=qT[0:Dh, h, qt * P : (qt + 1) * P],
                        rhs=kT[0:Dh, h, kc * QC : (kc + 1) * QC],
                        start=True,
                        stop=True,
                    )
                    if last:
                        nc.vector.tensor_reduce(
                            hm[:, pair : pair + 1],
                            st_t[:].rearrange("p a b -> p (a b)"),
                            axis=mybir.AxisListType.X, op=ALU.max,
                        )
                        if pair == 1:
                            nc.vector.tensor_reduce(
                                mx[:, qt4 : qt4 + 1], hm[:, 0:2],
                                axis=mybir.AxisListType.X, op=ALU.max,
                            )

                yield mm

    def drain(it, n=1 << 30):
        k = 0
        if it is not None:
            for step in it:
                step()
                k += 1
                if k >= n:
                    break

    def aug_steps(ps_pool, h, qc4):
        def step():
            mx = mx_tiles[(h, qc4)]
            psm = ps_pool.tile([P, QC], F32, tag="wo", name="psm")
            nc.tensor.transpose(psm[0:4, 0:P], mx[:, :], ident[:])
            mst = stage.tile([4, P], F32R, tag="mst")
            nc.scalar.mul(mst[:], psm[0:4, 0:P], -1.0)
            nc.gpsimd.dma_start(
                qT[Dh : Dh + 1, h, qc4 * QC : (qc4 + 1) * QC], mst[:, :]
            )
        yield step

    # ---------- W_o drain for one chunk ----------
    def wo_steps(ps_pools, qc4, alternate_ob=False):
        i = 0
        for qt4 in range(4):
            qt = qc4 * 4 + qt4
            for ec in range(E // QC):
                pool, ptag = ps_pools[i % len(ps_pools)]
                ps = pool.tile([P, QC], F32, tag=ptag, name="ps_wo")
                use_dve = alternate_ob and (i % 2 == 1)
                i += 1
                for fc in range(FG // P):

                    def mm(ps=ps, qt=qt, ec=ec, fc=fc, use_dve=use_dve,
                           last=(fc == FG // P - 1)):
                        nc.tensor.matmul(
                            ps,
                            lhsT=att[:, fc, qt * P : (qt + 1) * P],
                            rhs=wos[:, fc, ec * QC : (ec + 1) * QC],
                            start=(fc == 0),
                            stop=last,
                            skip_group_check=True,
                        )
                        if last:
                            ob = obp.tile([P, QC], F32, tag="ob")
                            if use_dve:
                                nc.vector.tensor_copy(ob[:], ps)
                            else:
                                nc.scalar.copy(ob[:], ps)
                            nc.sync.dma_start(
                                out[qt * P : (qt + 1) * P, ec * QC : (ec + 1) * QC],
                                ob[:],
                            )

                    yield mm

    slots = [(c, h) for c in range(NQC) for h in range(HPC)]
    NSLOT = len(slots)

    pp = ps_sc
    kproj(pp, 0, split_first=True)
    vproj(pp, 0)
    qproj(pp, 0)
    kproj(pp, 1, split_first=True)
    emit_x3()
    vproj(pp, 1)
    kproj(pp, 2)
    vproj(pp, 2)
    kproj(pp, 3)
    vproj(pp, 3)
    emit_wos()
    def chain(gens):
        for g in gens:
            yield from g

    # pre-slot: stats for slots 0 and 1, with Q1/Q2 projections as PE
    # filler while DVE grinds through the wide reduces (2-buf st pool)
    g0 = stats_steps(slots[0][1], slots[0][0], pair_major=True)
    g1 = stats_steps(slots[1][1], slots[1][0], pair_major=True)
    gq = chain([qproj_steps(ps_sc, 1), qproj_steps(ps_sc, 2),
                qproj_steps(ps_sc, 3)])
    for _ in range(8):
        drain(g0, 2)
        drain(gq, 3)
    for _ in range(8):
        drain(g1, 2)
        drain(gq, 3)
    drain(g0)
    drain(g1)
    drain(gq)

    # aug row for slot 0 must land before its first score matmul
    drain(aug_steps(ps_wo, slots[0][1], slots[0][0]))

    # fill lists per slot: [aug(s+1)] + stats(s+2) + wo drip
    fills = []
    for s in range(NSLOT):
        f = []
        aug_f = None
        if s + 1 < NSLOT:
            c1, h1 = slots[s + 1]
            aug_f = aug_steps(ps_wo, h1, c1)
        if s + 2 < NSLOT:
            c2, h2 = slots[s + 2]
            f.append(stats_steps(h2, c2))
        fills.append((aug_f, f))
    # wo drip: wo(c) spread over the 4 slots of group c+1
    wo_gens = {}
    for c in range(NQC - 1):
        wo_gens[c] = wo_steps([(ps_wo, "wo")], c)

    def norm_steps(pv, li, h, qs):
        # deferred normalize: pb broadcast matmul, Act copy to SBUF, DVE mult.
        # Drained a few kt into the NEXT slot so the pb matmul never waits on
        # the reciprocal at the head of PE's queue.
        def step():
            pb = ps_wo.tile([P, QC], F32, tag="wo", name="pb")
            nc.tensor.matmul(
                pb[0:Dh, :], lhsT=ones_mat[Dh : Dh + 1, :],
                rhs=li[Dh : Dh + 1, :], start=True, stop=True,
            )
            bc = stage2.tile([P, QC], F32, tag="bc")
            nc.scalar.copy(bc[0:Dh, :], pb[0:Dh, :])
            with nc.allow_low_precision(reason="normalize in f32r is ample"):
                if h % 2 == 0:
                    nc.vector.tensor_tensor(
                        att[0:Dh, h // 2, qs], pv[0:Dh, :], bc[0:Dh, :], ALU.mult
                    )
                else:
                    stg = stage2.tile([P, QC], F32R, tag="att_stg")
                    nc.vector.tensor_tensor(
                        stg[0:Dh, :], pv[0:Dh, :], bc[0:Dh, :], ALU.mult
                    )
                    nc.gpsimd.dma_start(
                        att[Dh : 2 * Dh, h // 2, qs], stg[0:Dh, :]
                    )
        yield step

    pending_norm = None
    for s, (qc4, h) in enumerate(slots):
        qs = slice(qc4 * QC, (qc4 + 1) * QC)
        aug_f, f_list = fills[s]
        fill = chain(f_list)
        aug_kt = 4 if s == 0 else 1
        # wo drip comes from the previous chunk's generator; for the first
        # slot of a group, delay until the previous group's last norm landed
        wo_g = wo_gens.get(qc4 - 1)
        wo_kts = (5, 9, 13, 15) if h == 0 else (1, 5, 9, 13)

        pv = ps_pv.tile([P, QC], F32, tag="pv")
        pts = {}

        def pv_mm(kt, pv=pv, h=h, pts=pts):
            nc.tensor.matmul(
                pv[0 : Dh + 1, :],
                lhsT=vau[:, kt, h, :],
                rhs=pts.pop(kt),
                start=(kt == 0),
                stop=(kt == ST - 1),
                skip_group_check=True,
            )

        sc2 = None
        for kt in range(ST):
            ks = slice(kt * P, (kt + 1) * P)
            if kt == 1 and pending_norm is not None:
                drain(pending_norm)
                pending_norm = None
            if kt == aug_kt and aug_f is not None:
                drain(aug_f)
            # stats fills strictly paced at 1 step/kt (DVE is the consumer)
            drain(fill, 1)
            if wo_g is not None and kt in wo_kts:
                drain(wo_g, 1)
            if kt % 2 == 0:
                sc2 = ps_sc.tile([P, 2, QC], F32, tag="sc")
            nc.tensor.matmul(
                sc2[:, kt % 2, :], lhsT=kT[0 : Dh + 1, h, ks],
                rhs=qT[0 : Dh + 1, h, qs], start=True, stop=True,
            )
            if kt % 2 == 1:
                pt2 = ptp.tile([P, 2, QC], F16, tag="pt")
                pts[kt - 1] = pt2[:, 0, :]
                pts[kt] = pt2[:, 1, :]
                nc.scalar.activation(pt2[:], sc2[:], AF.Exp, scale=SCALE)
            if kt >= LAG:
                pv_mm(kt - LAG)
        drain(fill)
        for kt in range(ST - LAG, ST):
            pv_mm(kt)

        li = stage2.tile([P, QC], F32R, tag="li")
        with nc.allow_low_precision(reason="1/l in f32r (~2^-12) is ample"):
            nc.vector.reciprocal(li[Dh : Dh + 1, :], pv[Dh : Dh + 1, :])
        pending_norm = norm_steps(pv, li, h, qs)

    # tail: last slot's norm, then the last chunk's W_o with ob copies
    # alternating Act/DVE (both engines idle by now)
    drain(pending_norm)
    drain(wo_steps([(ps_wo, "wo"), (ps_sc, "sc")], NQC - 1, alternate_ob=True))
    ctx.close()


_NC = None


def _build():
    global _NC
    if _NC is None:
        nc = bacc.Bacc(
            "TRN2", target_bir_lowering=False, debug=False, num_devices=NCORES
        )
        with tile.TileContext(nc) as tc:
            _emit_v2(tc)
        nc.compile()
        _NC = nc
    return _NC


def _prep_inputs(x, W_q, W_k, W_v, W_o):
    x = np.asarray(x, dtype=np.float32)
    W_q = np.asarray(W_q, dtype=np.float32)
    W_k = np.asarray(W_k, dtype=np.float32)
    W_v = np.asarray(W_v, dtype=np.float32)
    W_o = np.asarray(W_o, dtype=np.float32)

    xts = [np.ascontiguousarray(x[b].T) for b in range(B)]
    in_maps = []
    for c in range(NCORES):
        b, g = divmod(c, GROUPS)
        fg = slice(g * FG, (g + 1) * FG)
        in_maps.append(
            {
                "xt": xts[b],
                "wq": np.ascontiguousarray(W_q[:, fg]),
                "wk": np.ascontiguousarray(W_k[:, fg]),
                "wv": np.ascontiguousarray(W_v[:, fg]),
                "wo": np.ascontiguousarray(W_o[fg, :]),
            }
        )
    return in_maps


def run(inputs, **spmd_kwargs):
    nc = _build()
    in_maps = _prep_inputs(
        inputs["x"], inputs["W_q"], inputs["W_k"], inputs["W_v"], inputs["W_o"]
    )
    res = bass_utils.run_bass_kernel_spmd(
        nc, in_maps, core_ids=list(range(NCORES)), **spmd_kwargs
    )
    out = np.zeros((B, S, E), dtype=np.float32)
    for c in range(NCORES):
        out[c // GROUPS] += res.results[c]["out"]
    return out, res


def kernel(**inputs):
    out, _ = run(inputs)
    return out


# revision 29
# speedup vs baseline: 1.1300x; 1.0145x over previous
"""Multi-head attention (B=2, S=2048, E=1024, H=16, Dh=64) on 8 TRN2 NeuronCores.

Sharding: batch x head-group data/tensor parallel. Core c handles batch c//4
and heads [4*(c%4), 4*(c%4)+4): it computes Q/K/V projections for its 256
feature columns, full attention for its 4 heads, and a partial output
projection against its 256 rows of W_o. The host sums the 4 partials per
batch (the "all-reduce after W_o" step of the sharding hint, done at
unshard time) and concatenates the two batches.

Numerics (same as the validated v1): the pre-softmax path runs in float32r
(~2^-12 per-element input rounding, fp32 accumulate) at full PE rate. The
row max m comes from a q-major f32r score pass max-reduced on DVE; the
k-major score matmul subtracts m via an augmented contraction row (kT row
64 = 1, qT row 64 = -m), so exp() fuses the PSUM->SBUF copy on ScalarE
with scale=1/sqrt(Dh). The softmax denominator comes free from an appended
ones-column on V; normalization divides by the Pool-broadcast denominator
after the P@V matmul. P is fp16, V/att/W_o are f32r.

Schedule (v2, engine-balanced): PE is the roofline (~220us of matmuls);
every other engine is kept strictly below PE's per-slot pace so PE never
waits.
  - DVE runs ONLY the stats max-reduction, as wide [128,1024] PSUM reads
    (2 banks per reduce; ~9.8us per slot vs ~11us of PE work).
  - Pool (gpsimd) takes everything else elementwise: proj PSUM->SBUF
    staging copies, W_o output copies, the 1/l broadcast
    (partition_broadcast) and the normalize divide.
  - Slots iterate q-chunk OUTER, head INNER, so each chunk's W_o partial
    drains as fill work in the next chunk's slots (no W_o pileup at the
    tail).
  - Fills run two slots ahead (slot s drains stats matmuls for slot s+2
    and the aug-row transpose/DMA for slot s+1), so the aug DMA latency
    and the DVE reduce latency are fully hidden.
  - DMA order on the serialized DMA resource: wks, wvs, x0, x1, wqs, x2,
    x3, wos; kT/qT unstack DMAs ride the scalar queue, x/out ride sync,
    weights/aug ride the Pool SWDGE queue.
"""

from contextlib import ExitStack

import numpy as np

import concourse.bacc as bacc
import concourse.mybir as mybir
import concourse.tile as tile
from concourse import bass_utils
from concourse.masks import make_identity

AF = mybir.ActivationFunctionType
ALU = mybir.AluOpType
F32 = mybir.dt.float32
F16 = mybir.dt.float16
F32R = mybir.dt.float32r

B, S, E, H, Dh = 2, 2048, 1024, 16, 64
NCORES = 8
GROUPS = 4            # head groups (cores per batch)
HPC = H // GROUPS     # heads per core = 4
FG = HPC * Dh         # feature columns per core = 256
P = 128
SCALE = 1.0 / (Dh ** 0.5)

EO = E // P           # 8 contraction chunks
ST = S // P           # 16 sequence tiles of 128
QC = 512              # q-chunk width
NQC = S // QC         # 4
LAG = 5               # PV lags scores by LAG kt tiles


def _emit_v2(tc):
    nc = tc.nc
    xt = nc.dram_tensor("xt", [E, S], F32R, kind="ExternalInput").ap()
    wq = nc.dram_tensor("wq", [E, FG], F32R, kind="ExternalInput").ap()
    wk = nc.dram_tensor("wk", [E, FG], F32R, kind="ExternalInput").ap()
    wv = nc.dram_tensor("wv", [E, FG], F32R, kind="ExternalInput").ap()
    wo = nc.dram_tensor("wo", [FG, E], F32R, kind="ExternalInput").ap()
    out = nc.dram_tensor("out", [S, E], F32, kind="ExternalOutput").ap()

    ctx = ExitStack()
    const = ctx.enter_context(tc.tile_pool(name="const", bufs=1))
    persist = ctx.enter_context(tc.tile_pool(name="persist", bufs=1))
    stage = ctx.enter_context(tc.tile_pool(name="stage", bufs=3))
    xqp = ctx.enter_context(tc.tile_pool(name="xqp", bufs=3))
    stgp = ctx.enter_context(tc.tile_pool(name="stgp", bufs=3))
    ptp = ctx.enter_context(tc.tile_pool(name="ptp", bufs=4))
    obp = ctx.enter_context(tc.tile_pool(name="obp", bufs=4))
    stage2 = ctx.enter_context(tc.tile_pool(name="stage2", bufs=2))
    mxp = ctx.enter_context(tc.tile_pool(name="mxp", bufs=4))
    hmp = ctx.enter_context(tc.tile_pool(name="hmp", bufs=10))
    ps_sc = ctx.enter_context(tc.tile_pool(name="ps_sc", bufs=3, space="PSUM"))
    ps_pv = ctx.enter_context(tc.tile_pool(name="ps_pv", bufs=1, space="PSUM"))
    ps_wo = ctx.enter_context(tc.tile_pool(name="ps_wo", bufs=1, space="PSUM"))

    ones_f32 = const.tile([P, Dh], F32)
    nc.gpsimd.memset(ones_f32[:], 1.0)
    ones_mat = const.tile([P, Dh], F32R)
    nc.vector.tensor_copy(ones_mat[:], ones_f32[:])
    ident = const.tile([P, P], F32)
    make_identity(nc, ident[:])


    wqs = persist.tile([P, EO, FG], F32R)
    wks = persist.tile([P, EO, FG], F32R)
    wvs = persist.tile([P, EO, FG], F32R)
    wos = persist.tile([P, FG // P, E], F32R)
    qT = persist.tile([P, HPC, S], F32R)
    kT = persist.tile([P, HPC, S], F32R)
    vau = persist.tile([P, ST, HPC, Dh + 1], F16)
    att = persist.tile([P, FG // P, S], F32R)

    xt_re = xt.rearrange("(eo p) s -> p eo s", p=P)

    # DMA order on the serialized DMA resource: wks/x0 halves interleaved so
    # K proj starts ~6us in, then x1..x3 with just-in-time weights between.
    wk_re = wk.rearrange("(eo p) m -> p eo m", p=P)
    xq_tiles = {}
    for qc4 in range(3):
        xq_tiles[qc4] = xqp.tile([P, EO, QC], F32R, tag="xq", name=f"xq{qc4}")
    nc.sync.dma_start(wks[:, 0:4, :], wk_re[:, 0:4, :])
    nc.sync.dma_start(xq_tiles[0][:, :, 0:256], xt_re[:, :, 0:256])
    nc.sync.dma_start(wks[:, 4:, :], wk_re[:, 4:, :])
    nc.sync.dma_start(xq_tiles[0][:, :, 256:512], xt_re[:, :, 256:512])
    nc.sync.dma_start(wvs[:], wv.rearrange("(eo p) m -> p eo m", p=P))
    nc.sync.dma_start(xq_tiles[1][:, :, 0:256], xt_re[:, :, QC : QC + 256])
    nc.sync.dma_start(xq_tiles[1][:, :, 256:512], xt_re[:, :, QC + 256 : 2 * QC])
    nc.sync.dma_start(wqs[:], wq.rearrange("(eo p) m -> p eo m", p=P))
    xq3 = xqp.tile([P, EO, QC], F32R, tag="xq")
    xq_tiles[3] = xq3
    nc.sync.dma_start(xq_tiles[2][:], xt_re[:, :, 2 * QC : 3 * QC])


    def emit_x3():
        nc.sync.dma_start(xq3[:], xt_re[:, :, 3 * QC : 4 * QC])

    def emit_wos():
        nc.sync.dma_start(wos[:], wo.rearrange("(fo p) e -> p fo e", p=P))

    # PE p-state warmup: dummy matmuls on const data while the first x
    # chunk is in flight (results discarded; psum overwritten later).
    wmps = ps_sc.tile([P, 2, QC], F32, tag="sc", name="warm_ps")
    for _ in range(24):
        nc.tensor.matmul(
            wmps[0:Dh, 0, 0:Dh], lhsT=ones_mat[0:P, :],
            rhs=ones_f32[:].bitcast(F32R),
            start=True, stop=True, skip_group_check=True,
        )
    nc.gpsimd.memset(kT[Dh : Dh + 1, :, :].bitcast(F32), 1.0)
    nc.gpsimd.memset(vau[:, :, :, Dh : Dh + 1], 1.0)
    warm = stage.tile([P, 4], F32, tag="warm")
    nc.gpsimd.memset(warm[:], 0.0)
    warm2 = stage.tile([P, 4], F16, tag="warm2")
    nc.scalar.activation(warm2[:], warm[:], AF.Exp, scale=1.0)

    # ---------- projections ----------
    def kproj(pp, qc4, split_first=False):
        xq = xq_tiles[qc4]
        qs = slice(qc4 * QC, (qc4 + 1) * QC)
        for mc in range(FG // P):
            stg = stgp.tile([P, QC], F32R, tag="stg")
            if split_first and mc == 0:
                ps = pp.tile([P, QC], F32, tag="sc", name="ps_k0")[:, :256]
                for half in range(2):
                    cs = slice(half * 256, (half + 1) * 256)
                    for eo in range(EO):
                        nc.tensor.matmul(
                            ps,
                            lhsT=wks[:, eo, mc * P : (mc + 1) * P],
                            rhs=xq[:, eo, cs],
                            start=(eo == 0),
                            stop=(eo == EO - 1),
                        )
                    nc.vector.tensor_copy(stg[:, cs], ps)
            else:
                ps = pp.tile([P, QC], F32, tag="sc", name="ps_k")
                for eo in range(EO):
                    nc.tensor.matmul(
                        ps,
                        lhsT=wks[:, eo, mc * P : (mc + 1) * P],
                        rhs=xq[:, eo, :],
                        start=(eo == 0),
                        stop=(eo == EO - 1),
                    )
                nc.vector.tensor_copy(stg[:], ps)
            for hh in range(2):
                h = mc * 2 + hh
                nc.sync.dma_start(kT[0:Dh, h, qs], stg[hh * Dh : (hh + 1) * Dh, :])

    def vproj(pp, qc4):
        xq = xq_tiles[qc4]
        for st4 in range(4):
            st = qc4 * 4 + st4
            ps = pp.tile([P, QC], F32, tag="sc", name="ps_v")[:, :FG]
            for eo in range(EO):
                nc.tensor.matmul(
                    ps,
                    lhsT=xq[:, eo, st4 * P : (st4 + 1) * P],
                    rhs=wvs[:, eo, :],
                    start=(eo == 0),
                    stop=(eo == EO - 1),
                )
            nc.scalar.copy(
                vau[:, st, :, 0:Dh],
                ps.rearrange("p (h d) -> p h d", h=HPC),
            )

    def qproj(pp, qc4):
        for step in qproj_steps(pp, qc4):
            step()

    def qproj_steps(pp, qc4):
        xq = xq_tiles[qc4]
        qs = slice(qc4 * QC, (qc4 + 1) * QC)
        for mc in range(FG // P):
            ps = pp.tile([P, QC], F32, tag="sc", name="ps_q")
            for eo in range(EO):

                def mm(ps=ps, mc=mc, eo=eo, xq=xq, qs=qs, last=(eo == EO - 1)):
                    nc.tensor.matmul(
                        ps,
                        lhsT=wqs[:, eo, mc * P : (mc + 1) * P],
                        rhs=xq[:, eo, :],
                        start=(eo == 0),
                        stop=last,
                    )
                    if last:
                        stg = stgp.tile([P, QC], F32R, tag="stg")
                        nc.vector.tensor_copy(stg[:], ps)
                        for hh in range(2):
                            h = mc * 2 + hh
                            nc.sync.dma_start(
                                qT[0:Dh, h, qs], stg[hh * Dh : (hh + 1) * Dh, :]
                            )

                yield mm

    # ---------- stats (q-major max pass) ----------
    mx_tiles = {}

    def stats_steps(h, qc4, pair_major=False):
        """16 PE steps; DVE wide reduces attached; fills mx_tiles[(h,qc4)].

        pair_major=True emits all kc01 pairs before any kc23 pair, so the
        early reduces only depend on the first half of kT (overlaps the
        tail of the projection phase)."""
        mx = mxp.tile([P, 4], F32, tag="mx", name=f"mx{h}_{qc4}")
        mx_tiles[(h, qc4)] = mx
        hms = {}
        order = ([(qt4, pair) for pair in range(2) for qt4 in range(4)]
                 if pair_major else
                 [(qt4, pair) for qt4 in range(4) for pair in range(2)])
        for qt4, pair in order:
            qt = qc4 * 4 + qt4
            if qt4 not in hms:
                hms[qt4] = hmp.tile([P, 2], F32, tag="hm", name=f"hm{h}_{qc4}_{qt4}")
            hm = hms[qt4]
            st_t = ps_sc.tile([P, 2, QC], F32, tag="sc", name="ps_stat")
            for j in range(2):
                kc = pair * 2 + j

                def mm(st_t=st_t, j=j, kc=kc, h=h, qt=qt, pair=pair,
                       hm=hm, qt4=qt4, mx=mx, last=(j == 1)):
                    nc.tensor.matmul(
                        st_t[:, j, :],
                        lhsT=qT[0:Dh, h, qt * P : (qt + 1) * P],
                        rhs=kT[0:Dh, h, kc * QC : (kc + 1) * QC],
                        start=True,
                        stop=True,
                    )
                    if last:
                        nc.vector.tensor_reduce(
                            hm[:, pair : pair + 1],
                            st_t[:].rearrange("p a b -> p (a b)"),
                            axis=mybir.AxisListType.X, op=ALU.max,
                        )
                        if pair == 1:
                            nc.vector.tensor_reduce(
                                mx[:, qt4 : qt4 + 1], hm[:, 0:2],
                                axis=mybir.AxisListType.X, op=ALU.max,
                            )

                yield mm

    def drain(it, n=1 << 30):
        k = 0
        if it is not None:
            for step in it:
                step()
                k += 1
                if k >= n:
                    break

    def aug_steps(ps_pool, h, qc4):
        def step():
            mx = mx_tiles[(h, qc4)]
            psm = ps_pool.tile([P, QC], F32, tag="wo", name="psm")
            nc.tensor.transpose(psm[0:4, 0:P], mx[:, :], ident[:])
            mst = stage.tile([4, P], F32R, tag="mst")
            nc.scalar.mul(mst[:], psm[0:4, 0:P], -1.0)
            nc.gpsimd.dma_start(
                qT[Dh : Dh + 1, h, qc4 * QC : (qc4 + 1) * QC], mst[:, :]
            )
        yield step

    # ---------- W_o drain for one chunk ----------
    def wo_steps(ps_pools, qc4, alternate_ob=False):
        i = 0
        for qt4 in range(4):
            qt = qc4 * 4 + qt4
            for ec in range(E // QC):
                pool, ptag = ps_pools[i % len(ps_pools)]
                ps = pool.tile([P, QC], F32, tag=ptag, name="ps_wo")
                use_dve = alternate_ob and (i % 2 == 1)
                i += 1
                for fc in range(FG // P):

                    def mm(ps=ps, qt=qt, ec=ec, fc=fc, use_dve=use_dve,
                           last=(fc == FG // P - 1)):
                        nc.tensor.matmul(
                            ps,
                            lhsT=att[:, fc, qt * P : (qt + 1) * P],
                            rhs=wos[:, fc, ec * QC : (ec + 1) * QC],
                            start=(fc == 0),
                            stop=last,
                            skip_group_check=True,
                        )
                        if last:
                            ob = obp.tile([P, QC], F32, tag="ob")
                            if use_dve:
                                nc.vector.tensor_copy(ob[:], ps)
                            else:
                                nc.scalar.copy(ob[:], ps)
                            nc.sync.dma_start(
                                out[qt * P : (qt + 1) * P, ec * QC : (ec + 1) * QC],
                                ob[:],
                            )

                    yield mm

    slots = [(c, h) for c in range(NQC - 1) for h in range(HPC)]
    slots += [(NQC - 1, h) for h in (1, 3, 0, 2)]
    NSLOT = len(slots)

    pp = ps_sc
    kproj(pp, 0, split_first=True)
    vproj(pp, 0)
    qproj(pp, 0)
    kproj(pp, 1, split_first=True)
    emit_x3()
    vproj(pp, 1)
    kproj(pp, 2)
    vproj(pp, 2)
    kproj(pp, 3)
    vproj(pp, 3)
    emit_wos()
    def chain(gens):
        for g in gens:
            yield from g

    # pre-slot: stats for slots 0 and 1, with Q1/Q2 projections as PE
    # filler while DVE grinds through the wide reduces (2-buf st pool)
    g0 = stats_steps(slots[0][1], slots[0][0], pair_major=True)
    g1 = stats_steps(slots[1][1], slots[1][0], pair_major=True)
    gq = chain([qproj_steps(ps_sc, 1), qproj_steps(ps_sc, 2),
                qproj_steps(ps_sc, 3)])
    for _ in range(8):
        drain(g0, 2)
        drain(gq, 3)
    for _ in range(8):
        drain(g1, 2)
        drain(gq, 3)
    drain(g0)
    drain(g1)
    drain(gq)

    # aug row for slot 0 must land before its first score matmul
    drain(aug_steps(ps_wo, slots[0][1], slots[0][0]))

    # fill lists per slot: [aug(s+1)] + stats(s+2) + wo drip
    fills = []
    for s in range(NSLOT):
        f = []
        aug_f = None
        if s + 1 < NSLOT:
            c1, h1 = slots[s + 1]
            aug_f = aug_steps(ps_wo, h1, c1)
        if s + 2 < NSLOT:
            c2, h2 = slots[s + 2]
            f.append(stats_steps(h2, c2))
        fills.append((aug_f, f))
    # wo drip: wo(c) spread over the 4 slots of group c+1
    wo_gens = {}
    for c in range(NQC - 1):
        wo_gens[c] = wo_steps([(ps_wo, "wo")], c)

    def norm_steps(pv, li, h, qs):
        # deferred normalize: pb broadcast matmul, Act copy to SBUF, DVE mult.
        # Drained a few kt into the NEXT slot so the pb matmul never waits on
        # the reciprocal at the head of PE's queue.
        def step():
            pb = ps_wo.tile([P, QC], F32, tag="wo", name="pb")
            nc.tensor.matmul(
                pb[0:Dh, :], lhsT=ones_mat[Dh : Dh + 1, :],
                rhs=li[Dh : Dh + 1, :], start=True, stop=True,
            )
            bc = stage2.tile([P, QC], F32, tag="bc")
            nc.scalar.copy(bc[0:Dh, :], pb[0:Dh, :])
            with nc.allow_low_precision(reason="normalize in f32r is ample"):
                if h % 2 == 0:
                    nc.vector.tensor_tensor(
                        att[0:Dh, h // 2, qs], pv[0:Dh, :], bc[0:Dh, :], ALU.mult
                    )
                else:
                    stg = stage2.tile([P, QC], F32R, tag="att_stg")
                    nc.vector.tensor_tensor(
                        stg[0:Dh, :], pv[0:Dh, :], bc[0:Dh, :], ALU.mult
                    )
                    nc.gpsimd.dma_start(
                        att[Dh : 2 * Dh, h // 2, qs], stg[0:Dh, :]
                    )
        yield step

    pending_norm = None
    for s, (qc4, h) in enumerate(slots):
        qs = slice(qc4 * QC, (qc4 + 1) * QC)
        aug_f, f_list = fills[s]
        fill = chain(f_list)
        aug_kt = 4 if s == 0 else 1
        # wo drip comes from the previous chunk's generator; for the first
        # slot of a group, delay until the previous group's last norm landed
        wo_g = wo_gens.get(qc4 - 1)
        wo_kts = (5, 9, 13, 15) if h == 0 else (1, 5, 9, 13)

        pv = ps_pv.tile([P, QC], F32, tag="pv")
        pts = {}

        def pv_mm(kt, pv=pv, h=h, pts=pts):
            nc.tensor.matmul(
                pv[0 : Dh + 1, :],
                lhsT=vau[:, kt, h, :],
                rhs=pts.pop(kt),
                start=(kt == 0),
                stop=(kt == ST - 1),
                skip_group_check=True,
            )

        sc2 = None
        for kt in range(ST):
            ks = slice(kt * P, (kt + 1) * P)
            if kt == 1 and pending_norm is not None:
                drain(pending_norm)
                pending_norm = None
            if kt == aug_kt and aug_f is not None:
                drain(aug_f)
            # stats fills strictly paced at 1 step/kt (DVE is the consumer)
            drain(fill, 1)
            if wo_g is not None and kt in wo_kts:
                drain(wo_g, 1)
            if kt % 2 == 0:
                sc2 = ps_sc.tile([P, 2, QC], F32, tag="sc")
            nc.tensor.matmul(
                sc2[:, kt % 2, :], lhsT=kT[0 : Dh + 1, h, ks],
                rhs=qT[0 : Dh + 1, h, qs], start=True, stop=True,
            )
            if kt % 2 == 1:
                pt2 = ptp.tile([P, 2, QC], F16, tag="pt")
                pts[kt - 1] = pt2[:, 0, :]
                pts[kt] = pt2[:, 1, :]
                nc.scalar.activation(pt2[:], sc2[:], AF.Exp, scale=SCALE)
            if kt >= LAG:
                pv_mm(kt - LAG)
        drain(fill)
        for kt in range(ST - LAG, ST):
            pv_mm(kt)

        li = stage2.tile([P, QC], F32R, tag="li")
        with nc.allow_low_precision(reason="1/l in f32r (~2^-12) is ample"):
            nc.vector.reciprocal(li[Dh : Dh + 1, :], pv[Dh : Dh + 1, :])
        pending_norm = norm_steps(pv, li, h, qs)

    # tail: last slot's norm, then the last chunk's W_o.  fc0 chains (heads
    # 0/1, whose att rows landed a slot earlier) are issued first in waves of
    # four open PSUM accumulations, so PE stays busy while the last norm
    # drains; ob copies alternate Act/DVE.
    drain(pending_norm)
    tail_pools = [(ps_wo, "wo"), (ps_sc, "sc"), (ps_sc, "sc"), (ps_sc, "sc")]
    items = [(NQC - 1) * 4 + qt4 for qt4 in range(4)]
    pairs = [(qt, ec) for qt in items for ec in range(E // QC)]
    for wave in (pairs[:4], pairs[4:]):
        tiles = []
        for i, (qt, ec) in enumerate(wave):
            ps = tail_pools[i][0].tile([P, QC], F32, tag=tail_pools[i][1], name="ps_wt")
            tiles.append(ps)
            nc.tensor.matmul(
                ps, lhsT=att[:, 0, qt * P : (qt + 1) * P],
                rhs=wos[:, 0, ec * QC : (ec + 1) * QC],
                start=True, stop=False, skip_group_check=True,
            )
        for i, (qt, ec) in enumerate(wave):
            ps = tiles[i]
            nc.tensor.matmul(
                ps, lhsT=att[:, 1, qt * P : (qt + 1) * P],
                rhs=wos[:, 1, ec * QC : (ec + 1) * QC],
                start=False, stop=True, skip_group_check=True,
            )
            ob = obp.tile([P, QC], F32, tag="ob")
            if i % 2 == 1:
                nc.vector.tensor_copy(ob[:], ps)
            else:
                nc.scalar.copy(ob[:], ps)
            nc.sync.dma_start(
                out[qt * P : (qt + 1) * P, ec * QC : (ec + 1) * QC], ob[:]
            )
    ctx.close()


_NC = None


def _build():
    global _NC
    if _NC is None:
        nc = bacc.Bacc(
            "TRN2", target_bir_lowering=False, debug=False, num_devices=NCORES
        )
        with tile.TileContext(nc) as tc:
            _emit_v2(tc)
        nc.compile()
        _NC = nc
    return _NC


def _prep_inputs(x, W_q, W_k, W_v, W_o):
    x = np.asarray(x, dtype=np.float32)
    W_q = np.asarray(W_q, dtype=np.float32)
    W_k = np.asarray(W_k, dtype=np.float32)
    W_v = np.asarray(W_v, dtype=np.float32)
    W_o = np.asarray(W_o, dtype=np.float32)

    xts = [np.ascontiguousarray(x[b].T) for b in range(B)]
    in_maps = []
    for c in range(NCORES):
        b, g = divmod(c, GROUPS)
        fg = slice(g * FG, (g + 1) * FG)
        in_maps.append(
            {
                "xt": xts[b],
                "wq": np.ascontiguousarray(W_q[:, fg]),
                "wk": np.ascontiguousarray(W_k[:, fg]),
                "wv": np.ascontiguousarray(W_v[:, fg]),
                "wo": np.ascontiguousarray(W_o[fg, :]),
            }
        )
    return in_maps


def run(inputs, **spmd_kwargs):
    nc = _build()
    in_maps = _prep_inputs(
        inputs["x"], inputs["W_q"], inputs["W_k"], inputs["W_v"], inputs["W_o"]
    )
    res = bass_utils.run_bass_kernel_spmd(
        nc, in_maps, core_ids=list(range(NCORES)), **spmd_kwargs
    )
    out = np.zeros((B, S, E), dtype=np.float32)
    for c in range(NCORES):
        out[c // GROUPS] += res.results[c]["out"]
    return out, res


def kernel(**inputs):
    out, _ = run(inputs)
    return out
